# revision 19
# baseline (speedup 1.0000x reference)
"""Trainium2 Bass kernel for nn_Block_29085518528833 (PVT-style pooling
attention block + IRB conv-MLP).

Sharding: 8 cores = 4 batches x 2 token-halves. Each core processes one
batch's full image for the (tiny, replicated) pooling/kv path, and a
4992-token range (4608 own + 384 halo) for the token-parallel paths.
The host permutes tokens so every core's own range is rows [0, 4992) --
a single uniform SPMD program, no cross-core communication.

v2: fused attention-MLP group loop (C) with the IRB conv (E) interleaved
at a 2-group lag so the tensor engine never idles long enough for the
HAM clock gate to re-throttle.  t stays in an SBUF ring, residuals ride
the PSUM accumulations as identity matmuls, the depthwise conv runs 6
taps on PE + 3 taps on GPSIMD, and everything heavy is bf16.
"""

import sys

sys.path.insert(0, "/opt/trn_rl_repo")

from contextlib import ExitStack

import numpy as np
import ml_dtypes

import concourse.bass as bass
import concourse.bacc as bacc
import concourse.mybir as mybir
from concourse.tile import TileContext

FP = mybir.dt.float32
FR = mybir.dt.float32r
BF = mybir.dt.bfloat16
AF = mybir.ActivationFunctionType
ALU = mybir.AluOpType

B = 4
C = 512
NH = 8
HD = 64
HID = 2048
HIMG = 96
NTOK = HIMG * HIMG
EPS = 1e-5
OHS = [8, 6, 5, 4]
LS = [o * o for o in OHS]
LOFF = [0, 64, 100, 125]
L = 141
AREAS = [144, 256, 400, 576]
L32 = [64, 64, 32, 32]

HALF = NTOK // 2
HALO = 384
TRNG = HALF + HALO            # 4992
GRP = 384
NGRP = TRNG // GRP            # 13
NROWS = TRNG // HIMG          # 52
CCH = C // 128                # 4
MCH = HID // 128              # 16
NIMG_TILES = NTOK // 128      # 72
SCALE = HD ** (-0.5)
W = 4                         # t ring slots

TAPS = [(di, dj) for di in (-1, 0, 1) for dj in (-1, 0, 1)]
PE_TAPS = [(0, 0), (0, -1), (0, 1)] + \
    [(di, dj) for di in (-1, 1) for dj in (-1, 0, 1)]

LOFF176 = [0, 64, 128, 160]

_cache = {}


def _build_masks():
    M = np.zeros((NTOK, 176), np.float32)
    for s, oh in enumerate(OHS):
        sh = (np.arange(oh) * HIMG) // oh
        eh = -((-(np.arange(oh) + 1) * HIMG) // oh)
        for i in range(oh):
            for j in range(oh):
                hmask = np.zeros(HIMG, bool)
                hmask[sh[i]:eh[i]] = True
                wmask = np.zeros(HIMG, bool)
                wmask[sh[j]:eh[j]] = True
                tok = (hmask[:, None] & wmask[None, :]).reshape(-1)
                M[tok, LOFF176[s] + i * oh + j] = 1.0
    return M


def _conv_ranges(di, dj, nrows, ncols, lo_open, hi_open):
    oi0 = 0 if (di >= 0 or lo_open) else 1
    oi1 = nrows if (di <= 0 or hi_open) else nrows - 1
    oj0 = max(0, -dj)
    oj1 = ncols - max(0, dj)
    if oi1 <= oi0 or oj1 <= oj0:
        return None
    return oi0, oi1, oj0, oj1


def build_program():
    nc = bacc.Bacc("TRN2", target_bir_lowering=False, debug=False, num_devices=8)

    def din(name, shape, dtype=FP):
        return nc.dram_tensor(name, list(shape), dtype, kind="ExternalInput").ap()

    I = {}
    I["x"] = din("x", [NTOK, C])
    I["masks"] = din("masks", [NTOK, 176], BF)
    I["qgwT"] = din("qgwT", [C, C], BF)
    I["qb"] = din("qb", [128, CCH])
    I["kwT"] = din("kwT", [C, C], BF)
    I["vwT"] = din("vwT", [C, C], BF)
    I["projTp"] = din("projTp", [CCH, 128, C], BF)
    I["projb"] = din("projb", [1, C])
    I["fc1T6"] = din("fc1T6", [C, HID], BF)
    I["fc1bc"] = din("fc1bc", [128, MCH])
    I["diag6"] = din("diag6", [MCH, 9, 128, 128], BF)
    I["convb6"] = din("convb6", [128, MCH])
    I["fc2T6"] = din("fc2T6", [HID, C], BF)
    I["fc2b"] = din("fc2b", [1, C], BF)
    I["g1rep"] = din("g1rep", [128, C])
    I["abrep"] = din("abrep", [128, 4, C])
    I["agrep"] = din("agrep", [128, C], BF)
    I["btrep"] = din("btrep", [128, C], BF)
    I["poolw"] = din("poolw", [128, CCH, 4, 9])
    I["poolabd"] = din("poolabd", [128, CCH, 4])
    I["onescolA"] = din("onescolA", [128, 64], BF)
    I["onescolB"] = din("onescolB", [128, 24], BF)
    I["repmask2"] = din("repmask2", [8, 512])
    I["identF"] = din("identF", [128, 128])
    I["identB"] = din("identB", [128, 128], BF)
    I["onesr"] = din("onesr", [1, 128])

    out_dram = nc.dram_tensor("out", [TRNG, C], FP, kind="ExternalOutput").ap()
    x2_dram = nc.dram_tensor("x2_scratch", [TRNG, C], BF).ap()
    qT_dram = nc.dram_tensor("qT_scratch", [C, TRNG], BF).ap()

    with TileContext(nc) as tc:
        _program(nc, tc, I, out_dram, x2_dram, qT_dram)
    nc.compile()
    return nc


def _program(nc, tc, I, out_dram, x2_dram, qT_dram):
    ctx = ExitStack()
    with ctx:
        consts = ctx.enter_context(tc.tile_pool(name="consts", bufs=1))
        persist = ctx.enter_context(tc.tile_pool(name="persist", bufs=1))
        small = ctx.enter_context(tc.tile_pool(name="small", bufs=4))

        identF = consts.tile([128, 128], FR)
        nc.sync.dma_start(out=identF[:], in_=I["identF"].bitcast(FR))
        identB = consts.tile([128, 128], BF)
        nc.sync.dma_start(out=identB[:], in_=I["identB"])
        eps_t = consts.tile([128, 1], FP)
        nc.vector.memset(eps_t, EPS)
        halfb = consts.tile([128, 1], FP)
        nc.vector.memset(halfb, 0.5)
        ones1r = consts.tile([1, 128], FR)
        nc.sync.dma_start(out=ones1r[:], in_=I["onesr"].bitcast(FR))
        ones1b = consts.tile([1, 128], BF)
        nc.vector.memset(ones1b, 1.0)

        kT_sb = [persist.tile([128, 144], BF, name=f"kT{m}") for m in range(CCH)]
        Va_sb = persist.tile([128, C], BF)
        Vb_sb = persist.tile([128, C], BF)
        qb_sb = persist.tile([128, CCH], FP)
        nc.sync.dma_start(out=qb_sb[:], in_=I["qb"])
        projb_sb = persist.tile([1, C], FR)
        nc.sync.dma_start(out=projb_sb[:], in_=I["projb"].bitcast(FR))
        fc1bc = persist.tile([128, MCH], FP)
        nc.sync.dma_start(out=fc1bc[:], in_=I["fc1bc"])
        convb6 = persist.tile([128, MCH], FP)
        nc.sync.dma_start(out=convb6[:], in_=I["convb6"])
        fc2b_sb = persist.tile([1, C], BF)
        nc.sync.dma_start(out=fc2b_sb[:], in_=I["fc2b"])
        onescolA = persist.tile([128, 64], BF)
        nc.sync.dma_start(out=onescolA[:], in_=I["onescolA"])
        onescolB = persist.tile([128, 24], BF)
        nc.sync.dma_start(out=onescolB[:], in_=I["onescolB"])
        repmask2 = persist.tile([8, 512], FR)
        nc.sync.dma_start(out=repmask2[:], in_=I["repmask2"].bitcast(FR))

        def ln_factors(xt, p):
            # natural_log_exp table set only: rs = exp(-0.5*ln(var+eps))
            stats = small.tile([128, 6], FP, name="stats", tag="stats")
            nc.vector.bn_stats(out=stats[:p, :], in_=xt)
            mv = small.tile([128, 2], FP, name="mv", tag="mv")
            nc.vector.bn_aggr(out=mv[:p, :], in_=stats[:p, :])
            lnv = small.tile([128, 1], FP, name="lnv", tag="lnv")
            nc.scalar.activation(out=lnv[:p], in_=mv[:p, 1:2], func=AF.Ln,
                                 bias=eps_t[:p], scale=1.0)
            rs = small.tile([128, 1], FP, name="rs", tag="rs")
            nc.scalar.activation(out=rs[:p], in_=lnv[:p], func=AF.Exp,
                                 bias=0.0, scale=-0.5)
            nmurs = small.tile([128, 1], FP, name="nmurs", tag="nmurs")
            nc.vector.tensor_mul(nmurs[:p], mv[:p, 0:1], rs[:p])
            nc.vector.tensor_scalar_mul(nmurs[:p], nmurs[:p], -1.0)
            return rs, nmurs

        phAB = ExitStack()
        spool = phAB.enter_context(tc.tile_pool(name="spool", bufs=1))

        # ============ PHASE A: LN1, pool sums, q^T -> DRAM ============
        s_sb = []
        with ExitStack() as phA:
            strA = phA.enter_context(tc.tile_pool(name="strA", bufs=3))
            wA = phA.enter_context(tc.tile_pool(name="wA", bufs=1))
            qgwT = [wA.tile([128, C], BF, name=f"qgwT{m}") for m in range(CCH)]
            for m in range(CCH):
                nc.sync.dma_start(out=qgwT[m][:],
                                  in_=I["qgwT"][m * 128:(m + 1) * 128, :])
            psA = phA.enter_context(tc.tile_pool(name="psA", bufs=1, space="PSUM"))
            spsA = psA.tile([128, C], FP, name="spsA")
            spsB = psA.tile([48, C], FP, name="spsB")
            sps = [spsA[0:64], spsA[64:100], spsB[0:25], spsB[32:48]]
            psT = phA.enter_context(tc.tile_pool(name="psT", bufs=3, space="PSUM"))
            psQ = phA.enter_context(tc.tile_pool(name="psQ", bufs=3, space="PSUM"))
            xTg = phA.enter_context(tc.tile_pool(name="xTg", bufs=2))
            qstg = phA.enter_context(tc.tile_pool(name="qstg", bufs=4))

            xT_cur = None
            for ti in range(NIMG_TILES):
                xt = strA.tile([128, C], FP, name="xt", tag="xt")
                nc.sync.dma_start(out=xt[:], in_=I["x"][ti * 128:(ti + 1) * 128, :])
                mt = strA.tile([128, 176], BF, name="mt", tag="mt")
                nc.sync.dma_start(out=mt[:],
                                  in_=I["masks"][ti * 128:(ti + 1) * 128, :])
                rs, nmurs = ln_factors(xt[:], 128)
                xh = strA.tile([128, C], BF, name="xh", tag="xh")
                if ti < TRNG // 128:
                    nc.scalar.activation(out=xh[:], in_=xt[:], func=AF.Identity,
                                         bias=nmurs[:], scale=rs[:])
                else:
                    nc.vector.tensor_scalar(out=xh[:], in0=xt[:], scalar1=rs[:],
                                            scalar2=nmurs[:], op0=ALU.mult,
                                            op1=ALU.add)
                nc.tensor.matmul(spsA[:], mt[:, 0:128], xh[:],
                                 start=(ti == 0), stop=(ti == NIMG_TILES - 1))
                nc.tensor.matmul(spsB[:], mt[:, 128:176], xh[:],
                                 start=(ti == 0), stop=(ti == NIMG_TILES - 1))
                if ti < TRNG // 128:
                    gi, sub = divmod(ti, 3)
                    if sub == 0:
                        xT_cur = xTg.tile([128, 3 * C], BF, name="xT1",
                                          tag="xT1")
                    tp = psT.tile([128, C], BF, name="tpA", tag="tpA")
                    for cc in range(CCH):
                        nc.tensor.transpose(tp[:, cc * 128:(cc + 1) * 128],
                                            xh[:, cc * 128:(cc + 1) * 128],
                                            identB[:])
                    nc.vector.tensor_copy(
                        out=xT_cur[:, sub * C:(sub + 1) * C], in_=tp[:])
                    if sub == 2:
                        xT3 = xT_cur[:].rearrange("p (s c) -> p s c", s=3)
                        for m in range(CCH):
                            qp = psQ.tile([128, GRP], FP, name="qp", tag="qp")
                            for cc in range(CCH):
                                nc.tensor.matmul(
                                    qp[:].rearrange("p (s f) -> p s f", s=3),
                                    qgwT[cc][:, m * 128:(m + 1) * 128],
                                    xT3[:, :, cc * 128:(cc + 1) * 128],
                                    start=(cc == 0), stop=(cc == CCH - 1))
                            qs = qstg.tile([128, GRP], BF, name="qs", tag="qs")
                            nc.scalar.activation(
                                out=qs[:], in_=qp[:],
                                func=AF.Identity, bias=qb_sb[:, m:m + 1],
                                scale=1.0)
                            nc.sync.dma_start(
                                out=qT_dram[m * 128:(m + 1) * 128,
                                            gi * GRP:(gi + 1) * GRP],
                                in_=qs[:])

            SBASE = [0, 64, 0, 32]
            for s in range(4):
                t = spool.tile([L32[s] + SBASE[s], C], FR, name=f"ssb{s}")
                nc.scalar.copy(t[SBASE[s]:SBASE[s] + LS[s], :], sps[s][:])
                s_sb.append(t)

        # ============ PHASE B: pool dwconv + attn LN + k/v ============
        with ExitStack() as phB:
            wB = phB.enter_context(tc.tile_pool(name="wB", bufs=1))
            kwT = [wB.tile([128, C], BF, name=f"kwT{m}") for m in range(CCH)]
            vwT = [wB.tile([128, C], BF, name=f"vwT{m}") for m in range(CCH)]
            for m in range(CCH):
                nc.sync.dma_start(out=kwT[m][:],
                                  in_=I["kwT"][m * 128:(m + 1) * 128, :])
                nc.sync.dma_start(out=vwT[m][:],
                                  in_=I["vwT"][m * 128:(m + 1) * 128, :])
            g1rep = wB.tile([128, C], FP, name="g1rep")
            nc.sync.dma_start(out=g1rep[:], in_=I["g1rep"])
            abrep = wB.tile([128, 4, C], FP, name="abrep")
            nc.sync.dma_start(out=abrep[:], in_=I["abrep"])
            agrep = wB.tile([128, C], BF, name="agrep")
            nc.sync.dma_start(out=agrep[:], in_=I["agrep"])
            btrep = wB.tile([128, C], BF, name="btrep")
            nc.sync.dma_start(out=btrep[:], in_=I["btrep"])
            poolw = wB.tile([128, CCH, 4, 9], FP, name="poolw")
            nc.sync.dma_start(out=poolw[:], in_=I["poolw"])
            poolabd = wB.tile([128, CCH, 4], FP, name="poolabd")
            nc.sync.dma_start(out=poolabd[:], in_=I["poolabd"])

            psB = phB.enter_context(tc.tile_pool(name="psB", bufs=2, space="PSUM"))
            sbB = phB.enter_context(tc.tile_pool(name="sbB", bufs=2))
            accB = phB.enter_context(tc.tile_pool(name="accB", bufs=1))
            rkv = phB.enter_context(tc.tile_pool(name="rkv", bufs=1))
            rhskv = [rkv.tile([128, 144], BF, name=f"rhskv{m}")
                     for m in range(CCH)]

            SBASE = [0, 64, 0, 32]
            for s in range(4):
                b0 = SBASE[s]
                nc.vector.tensor_mul(s_sb[s][b0:b0 + LS[s], :],
                                     s_sb[s][b0:b0 + LS[s], :],
                                     g1rep[b0:b0 + LS[s], :])
                nc.vector.tensor_add(s_sb[s][b0:b0 + LS[s], :],
                                     s_sb[s][b0:b0 + LS[s], :],
                                     abrep[b0:b0 + LS[s], s, :])

            pn = []
            for s in range(4):
                oh = OHS[s]
                s1T = [sbB.tile([128, LS[s]], FR, name=f"s1T{s}_{cc}",
                                tag=f"s1T{cc}") for cc in range(CCH)]
                b0 = SBASE[s]
                for cc in range(CCH):
                    tp = psB.tile([128, 512], FR, name="tpB", tag="pb")
                    nc.tensor.transpose(tp[:, 0:L32[s]],
                                        s_sb[s][b0:b0 + L32[s],
                                                cc * 128:(cc + 1) * 128],
                                        identF[b0:b0 + L32[s], b0:b0 + L32[s]])
                    nc.scalar.copy(s1T[cc][:], tp[:, 0:LS[s]])
                acc = [accB.tile([128, L32[s]], FR, name=f"acc{s}_{cc}",
                                 tag=f"acc{cc}") for cc in range(CCH)]
                for cc in range(CCH):
                    nc.vector.tensor_scalar_add(acc[cc][:, 0:LS[s]], s1T[cc][:],
                                                poolabd[:, cc, s:s + 1])
                    for tap, (di, dj) in enumerate(TAPS):
                        r = _conv_ranges(di, dj, oh, oh, False, False)
                        if r is None:
                            continue
                        oi0, oi1, oj0, oj1 = r
                        o_ap = acc[cc][:, 0:LS[s]].rearrange(
                            "p (i j) -> p i j", i=oh)[:, oi0:oi1, oj0:oj1]
                        i_ap = s1T[cc][:].rearrange("p (i j) -> p i j", i=oh)[
                            :, oi0 + di:oi1 + di, oj0 + dj:oj1 + dj]
                        nc.vector.scalar_tensor_tensor(
                            out=o_ap, in0=i_ap, scalar=poolw[:, cc, s, tap:tap + 1],
                            in1=o_ap, op0=ALU.mult, op1=ALU.add)
                q_s = sbB.tile([LS[s], C], FP, name=f"q_s{s}", tag="q_s")
                for cc in range(CCH):
                    tp = psB.tile([128, 512], FR, name="tpB2", tag="pb")
                    nc.tensor.transpose(tp[0:L32[s], 0:128], acc[cc][:],
                                        identF[:])
                    nc.scalar.copy(q_s[:, cc * 128:(cc + 1) * 128],
                                   tp[0:LS[s], 0:128])
                rs, nmurs = ln_factors(q_s[:], LS[s])
                pn_s = sbB.tile([L32[s], C], BF, name=f"pn{s}", tag="pn_s")
                nc.scalar.activation(out=pn_s[0:LS[s], :], in_=q_s[:],
                                     func=AF.Identity,
                                     bias=nmurs[0:LS[s]], scale=rs[0:LS[s]])
                nc.vector.tensor_mul(pn_s[0:LS[s], :], pn_s[0:LS[s], :],
                                     agrep[0:LS[s], :])
                nc.vector.tensor_add(pn_s[0:LS[s], :], pn_s[0:LS[s], :],
                                     btrep[0:LS[s], :])
                pn.append(pn_s)

            for s in range(4):
                for cc in range(CCH):
                    tp = psB.tile([128, 512], BF, name="tpB3", tag="pb")
                    nc.tensor.transpose(tp[:, 0:L32[s]],
                                        pn[s][:, cc * 128:(cc + 1) * 128],
                                        identB[0:L32[s], 0:L32[s]])
                    nc.scalar.copy(rhskv[cc][:, LOFF[s]:LOFF[s] + LS[s]],
                                   tp[:, 0:LS[s]])

            for m in range(CCH):
                kp = psB.tile([128, 144], FP, name="kp", tag="pb")
                for cc in range(CCH):
                    nc.tensor.matmul(kp[:], kwT[cc][:, m * 128:(m + 1) * 128],
                                     rhskv[cc][:], start=(cc == 0),
                                     stop=(cc == CCH - 1))
                nc.scalar.copy(kT_sb[m][:], kp[:])
            vp = psB.tile([128, C], FP, name="vp", tag="pb")
            for cc in range(CCH):
                nc.tensor.matmul(vp[:], rhskv[cc][:, 0:128], vwT[cc][:],
                                 start=(cc == 0), stop=(cc == CCH - 1))
            nc.scalar.copy(Va_sb[:], vp[:])
            vp2 = psB.tile([13, C], FP, name="vp2", tag="pb")
            for cc in range(CCH):
                nc.tensor.matmul(vp2[:], rhskv[cc][:, 128:L], vwT[cc][:],
                                 start=(cc == 0), stop=(cc == CCH - 1))
            vbtmp = sbB.tile([13, C], BF, name="vbtmp", tag="vbtmp")
            nc.scalar.copy(vbtmp[:], vp2[:])
            for hh in range(4):
                nc.sync.dma_start(out=Vb_sb[32 * hh:32 * hh + 13, :],
                                  in_=vbtmp[:])
        phAB.close()

        # ============ PHASE CE: fused attention+fc1 / dwconv+fc2 ============
        with ExitStack() as phC:
            wC = phC.enter_context(tc.tile_pool(name="wC", bufs=1))
            fc1T = [wC.tile([128, HID], BF, name=f"fc1T{cc}") for cc in range(CCH)]
            for cc in range(CCH):
                nc.sync.dma_start(out=fc1T[cc][:],
                                  in_=I["fc1T6"][cc * 128:(cc + 1) * 128, :])
            projTp = [wC.tile([128, C], BF, name=f"projTp{p}") for p in range(4)]
            for p in range(4):
                nc.sync.dma_start(out=projTp[p][:], in_=I["projTp"][p])
            fc2T = [wC.tile([128, C], BF, name=f"fc2T{m}") for m in range(MCH)]
            for m in range(MCH):
                nc.sync.dma_start(out=fc2T[m][:],
                                  in_=I["fc2T6"][m * 128:(m + 1) * 128, :])
            diag6 = [wC.tile([128, 9, 128], BF, name=f"diag6_{m}")
                     for m in range(MCH)]
            for m in range(MCH):
                src = bass.AP(tensor=I["diag6"].tensor,
                              offset=I["diag6"].offset + m * 9 * 128 * 128,
                              ap=[[128, 128], [128 * 128, 9], [1, 128]])
                nc.sync.dma_start(out=diag6[m][:], in_=src)

            ringP = phC.enter_context(tc.tile_pool(name="ringP", bufs=1))
            ring = [ringP.tile([128, W * GRP], BF, name=f"ring{m}")
                    for m in range(MCH)]
            t2P = phC.enter_context(tc.tile_pool(name="t2P", bufs=1))
            t2t = [t2P.tile([128, GRP], BF, name=f"t2_{m}") for m in range(MCH)]

            psM = phC.enter_context(tc.tile_pool(name="psM", bufs=4, space="PSUM"))
            psDw = phC.enter_context(tc.tile_pool(name="psDw", bufs=2,
                                                  space="PSUM"))
            psOp = phC.enter_context(tc.tile_pool(name="psOp", bufs=2,
                                                  space="PSUM"))

            qTs = phC.enter_context(tc.tile_pool(name="qTs", bufs=2))
            EaP = phC.enter_context(tc.tile_pool(name="EaP", bufs=1))
            EbP = phC.enter_context(tc.tile_pool(name="EbP", bufs=1))
            AhP = phC.enter_context(tc.tile_pool(name="AhP", bufs=1))
            sbT = phC.enter_context(tc.tile_pool(name="sbT", bufs=1))
            strX = phC.enter_context(tc.tile_pool(name="strX", bufs=2))
            strXt = phC.enter_context(tc.tile_pool(name="strXt", bufs=2))
            x2P = phC.enter_context(tc.tile_pool(name="x2P", bufs=2))
            strE = phC.enter_context(tc.tile_pool(name="strE", bufs=3))
            osbP = phC.enter_context(tc.tile_pool(name="osbP", bufs=2))
            rcP = phC.enter_context(tc.tile_pool(name="rcP", bufs=1))

            qdict, xdict, x2dict = {}, {}, {}

            def prefetch_qT(g):
                g0 = g * GRP
                qTg = [qTs.tile([128, GRP], BF, name=f"qTg{m}", tag=f"qTg{m}")
                       for m in range(CCH)]
                for m in range(CCH):
                    nc.sync.dma_start(
                        out=qTg[m][:],
                        in_=qT_dram[m * 128:(m + 1) * 128, g0:g0 + GRP])
                qdict[g] = qTg

            def prefetch_xt(g):
                g0 = g * GRP
                xts = []
                for sub in range(3):
                    xt = strXt.tile([128, C], FR, name=f"xt{sub}",
                                    tag=f"xt{sub}")
                    nc.sync.dma_start(
                        out=xt[:], in_=I["x"][g0 + sub * 128:
                                              g0 + (sub + 1) * 128,
                                              :].bitcast(FR))
                    xts.append(xt)
                xdict[g] = xts

            def prefetch_x2(gp):
                g0 = gp * GRP
                x2s = []
                for sub in range(3):
                    x2t = x2P.tile([128, C], BF, name=f"x2t{sub}",
                                   tag=f"x2t{sub}")
                    nc.sync.dma_start(
                        out=x2t[:], in_=x2_dram[g0 + sub * 128:
                                                g0 + (sub + 1) * 128, :])
                    x2s.append(x2t)
                x2dict[gp] = x2s

            def emit_attn_S(g):
                qTg = qdict.pop(g)
                Ea = []
                for h in range(NH):
                    m, hh = h // 2, (h % 2) * 64
                    Sa = psM.tile([128, GRP], FP, name="Sa", tag="pc")
                    nc.tensor.matmul(Sa[:], kT_sb[m][hh:hh + 64, 0:128],
                                     qTg[m][hh:hh + 64, :],
                                     start=True, stop=True)
                    Eh = EaP.tile([128, GRP], BF, name=f"Ea{h}", tag=f"Ea{h}")
                    nc.scalar.activation(out=Eh[:], in_=Sa[:], func=AF.Exp)
                    Ea.append(Eh)
                Eb = []
                for t in range(3):
                    heads = range(3 * t, min(3 * t + 3, NH))
                    hi = 32 * (len(list(heads)) - 1) + 13
                    SbP = psM.tile([128, GRP], FP, name="Sb", tag="pc")
                    nc.vector.memset(SbP[:], 0.0)
                    for h in heads:
                        hq = h - 3 * t
                        m, hh = h // 2, (h % 2) * 64
                        nc.tensor.matmul(SbP[32 * hq:32 * hq + 13, :],
                                         kT_sb[m][hh:hh + 64, 128:L],
                                         qTg[m][hh:hh + 64, :],
                                         start=True, stop=True)
                    Et = EbP.tile([128, GRP], BF, name=f"Eb{t}", tag=f"Eb{t}")
                    nc.scalar.activation(out=Et[0:hi, :], in_=SbP[0:hi, :],
                                         func=AF.Exp)
                    Eb.append(Et)
                return Ea, Eb

            def emit_attn_tail(g, Ea, Eb):
                den = psM.tile([8, GRP], FP, name="den", tag="pc")
                for h in range(NH):
                    nc.tensor.matmul(den[:], onescolA[:, h * 8:h * 8 + 8],
                                     Ea[h][:], start=(h == 0), stop=False)
                for t in range(3):
                    hi = 77 if t < 2 else 45
                    nc.tensor.matmul(den[:], onescolB[0:hi, 8 * t:8 * t + 8],
                                     Eb[t][0:hi, :], start=False, stop=(t == 2))
                recip = rcP.tile([8, GRP], FR, name="recip", tag="recip")
                with nc.allow_low_precision("f32r reciprocal feeds matmul"):
                    nc.vector.reciprocal(recip[:], den[:])
                Ah = []
                for p in range(4):
                    U = psM.tile([128, GRP], FP, name="U", tag="pc")
                    for half in range(2):
                        h = 2 * p + half
                        hq = h % 3
                        o0 = 64 * half
                        nc.tensor.matmul(U[o0:o0 + 64, :],
                                         Va_sb[:, h * 64:(h + 1) * 64],
                                         Ea[h][:], start=True, stop=False)
                        nc.tensor.matmul(U[o0:o0 + 64, :],
                                         Vb_sb[32 * hq:32 * hq + 13,
                                               h * 64:(h + 1) * 64],
                                         Eb[h // 3][32 * hq:32 * hq + 13, :],
                                         start=False, stop=True)
                    rr = psM.tile([128, GRP], FP, name="rr", tag="pc")
                    nc.tensor.matmul(rr[:], repmask2[:, p * 128:(p + 1) * 128],
                                     recip[:], start=True, stop=True)
                    rrs = strX.tile([128, GRP], BF, name="rrs", tag="rrs")
                    nc.scalar.copy(rrs[:], rr[:])
                    a = AhP.tile([128, GRP], BF, name=f"Ah{p}", tag=f"Ah{p}")
                    nc.vector.tensor_mul(a[:], U[:], rrs[:])
                    Ah.append(a)
                return Ah

            def emit_proj_ln(g, Ah):
                g0 = g * GRP
                xts = xdict.pop(g)
                xh2T = sbT.tile([128, 3 * C], BF, name="xh2T", tag="xh2T")
                for sub in range(3):
                    r0 = g0 + sub * 128
                    xt = xts[sub]
                    xp = psM.tile([128, C], FP, name="xp", tag="pc")
                    for p in range(4):
                        nc.tensor.matmul(xp[:],
                                         Ah[p][:, sub * 128:(sub + 1) * 128],
                                         projTp[p][:], start=(p == 0),
                                         stop=False)
                    nc.tensor.matmul(xp[:], ones1r[:], projb_sb[:],
                                     start=False, stop=False)
                    nc.tensor.matmul(xp[:], identF[:], xt[:],
                                     start=False, stop=True)
                    rs, nmurs = ln_factors(xp[:], 128)
                    x2b = strX.tile([128, C], BF, name="x2b", tag="x2b")
                    nc.scalar.copy(x2b[:], xp[:])
                    nc.sync.dma_start(out=x2_dram[r0:r0 + 128, :], in_=x2b[:])
                    xh2 = strX.tile([128, C], BF, name="xh2", tag="xh2")
                    nc.scalar.activation(out=xh2[:], in_=xp[:], func=AF.Identity,
                                         bias=nmurs[:], scale=rs[:])
                    tp = psM.tile([128, C], BF, name="tpC", tag="pc")
                    for cc in range(CCH):
                        nc.tensor.transpose(tp[:, cc * 128:(cc + 1) * 128],
                                            xh2[:, cc * 128:(cc + 1) * 128],
                                            identB[:])
                    nc.vector.tensor_copy(out=xh2T[:, sub * C:(sub + 1) * C],
                                          in_=tp[:])
                return xh2T

            def emit_fc1(g, xh2T):
                slot = g % W
                x3 = xh2T[:].rearrange("p (s c) -> p s c", s=3)
                for m in range(MCH):
                    fp = psM.tile([128, GRP], FP, name="fp", tag="pc")
                    for cc in range(CCH):
                        nc.tensor.matmul(
                            fp[:].rearrange("p (s f) -> p s f", s=3),
                            fc1T[cc][:, m * 128:(m + 1) * 128],
                            x3[:, :, cc * 128:(cc + 1) * 128],
                            start=(cc == 0), stop=(cc == CCH - 1))
                    c1 = strX.tile([128, GRP], BF, name="c1", tag="c1")
                    nc.scalar.activation(out=c1[:], in_=fp[:], func=AF.Relu,
                                         bias=fc1bc[:, m:m + 1], scale=1.0)
                    up = strX.tile([128, GRP], BF, name="up", tag="up")
                    nc.vector.tensor_scalar_add(up[:], c1[:], -0.5)
                    nc.vector.scalar_tensor_tensor(
                        out=ring[m][:, slot * GRP:(slot + 1) * GRP],
                        in0=c1[:], scalar=1.0, in1=up[:],
                        op0=ALU.min, op1=ALU.mult)

            def emit_dw_hswish(gp):
                # PE taps (rows +-1) + merge of gpsimd acc; then hswish2
                for m in range(MCH):
                    dw = psDw.tile([128, GRP], FP, name="dw", tag="dw")
                    dw3 = dw[:].rearrange("p (i j) -> p i j", i=4)
                    pieces = []
                    for t_i, (di, dj) in enumerate(PE_TAPS):
                        oj0, oj1 = max(0, -dj), HIMG - max(0, dj)
                        if di == 0:
                            pieces.append((t_i, 0, 4, gp % W, 0, oj0, oj1, dj))
                        elif di == -1:
                            if gp > 0:
                                pieces.append((t_i, 0, 1, (gp - 1) % W, 3,
                                               oj0, oj1, dj))
                            pieces.append((t_i, 1, 4, gp % W, 0, oj0, oj1, dj))
                        else:
                            pieces.append((t_i, 0, 3, gp % W, 1, oj0, oj1, dj))
                            if gp < NGRP - 1:
                                pieces.append((t_i, 3, 4, (gp + 1) % W, 0,
                                               oj0, oj1, dj))
                    for n, (t_i, r0, r1, slot, sr0, oj0, oj1, dj) in \
                            enumerate(pieces):
                        r3 = ring[m][:, slot * GRP:(slot + 1) * GRP].rearrange(
                            "p (i j) -> p i j", i=4)
                        nc.tensor.matmul(
                            dw3[:, r0:r1, oj0:oj1],
                            diag6[m][:, t_i, :],
                            r3[:, sr0:sr0 + (r1 - r0), oj0 + dj:oj1 + dj],
                            start=(n == 0), stop=(n == len(pieces) - 1))
                    c2 = strE.tile([128, GRP], BF, name="c2", tag="c2")
                    nc.scalar.activation(out=c2[:], in_=dw[:], func=AF.Relu,
                                         bias=convb6[:, m:m + 1], scale=1.0)
                    u2 = strE.tile([128, GRP], BF, name="u2", tag="u2")
                    nc.vector.tensor_scalar_add(u2[:], c2[:], -0.5)
                    nc.vector.scalar_tensor_tensor(
                        out=t2t[m][:], in0=c2[:], scalar=1.0, in1=u2[:],
                        op0=ALU.min, op1=ALU.mult)

            def emit_fc2(gp):
                g0 = gp * GRP
                x2s = x2dict.pop(gp)
                for sub in range(3):
                    r0 = g0 + sub * 128
                    x2t = x2s[sub]
                    op = psOp.tile([128, C], FP, name="op", tag="op")
                    for m in range(MCH):
                        nc.tensor.matmul(op[:],
                                         t2t[m][:, sub * 128:(sub + 1) * 128],
                                         fc2T[m][:], start=(m == 0), stop=False)
                    nc.tensor.matmul(op[:], ones1b[:], fc2b_sb[:],
                                     start=False, stop=False)
                    nc.tensor.matmul(op[:], identB[:], x2t[:],
                                     start=False, stop=True)
                    osb = osbP.tile([128, C], FP, name="osb", tag="osb")
                    nc.scalar.copy(osb[:], op[:])
                    nc.sync.dma_start(out=out_dram[r0:r0 + 128, :], in_=osb[:])

            prefetch_qT(0)
            prefetch_xt(0)
            for g in range(NGRP + 2):
                if g + 1 < NGRP:
                    prefetch_qT(g + 1)
                    prefetch_xt(g + 1)
                if 0 <= g - 1 < NGRP:
                    prefetch_x2(g - 1)
                if g < NGRP:
                    Ea, Eb = emit_attn_S(g)
                if g >= 2:
                    emit_dw_hswish(g - 2)
                if g < NGRP:
                    Ah = emit_attn_tail(g, Ea, Eb)
                    xh2T = emit_proj_ln(g, Ah)
                    emit_fc1(g, xh2T)
                if g >= 2:
                    emit_fc2(g - 2)


def _host_prep(inputs):
    x = np.asarray(inputs["x"], np.float32)
    g1 = np.asarray(inputs["norm1_g"], np.float32)
    b1 = np.asarray(inputs["norm1_b"], np.float32)
    q_w = np.asarray(inputs["q_w"], np.float32)
    kv_w = np.asarray(inputs["kv_w"], np.float32)
    ag = np.asarray(inputs["attn_norm_g"], np.float32)
    ab = np.asarray(inputs["attn_norm_b"], np.float32)
    proj_w = np.asarray(inputs["proj_w"], np.float32)
    proj_b = np.asarray(inputs["proj_b"], np.float32)
    dconv_w = np.asarray(inputs["dconv_w"], np.float32)
    dconv_b = np.asarray(inputs["dconv_b"], np.float32)
    g2 = np.asarray(inputs["norm2_g"], np.float32)
    b2 = np.asarray(inputs["norm2_b"], np.float32)
    fc1_w = np.asarray(inputs["fc1_w"], np.float32)
    fc1_b = np.asarray(inputs["fc1_b"], np.float32)
    conv_w = np.asarray(inputs["conv_w"], np.float32)
    conv_b = np.asarray(inputs["conv_b"], np.float32)
    fc2_w = np.asarray(inputs["fc2_w"], np.float32)
    fc2_b = np.asarray(inputs["fc2_b"], np.float32)

    BFD = ml_dtypes.bfloat16
    M = _build_masks().astype(BFD)

    qgw = (q_w * g1[None, :]) * SCALE
    qgwT = np.ascontiguousarray(qgw.T).astype(BFD)
    qb = np.ascontiguousarray(((q_w @ b1) * SCALE).reshape(CCH, 128).T)
    kwT = np.ascontiguousarray(kv_w[0:C].T).astype(BFD)
    vwT = np.ascontiguousarray(kv_w[C:2 * C].T).astype(BFD)
    projTp = np.ascontiguousarray(proj_w.T.reshape(CCH, 128, C)).astype(BFD)
    projb = proj_b.reshape(1, C)
    fc1T6 = np.ascontiguousarray(((fc1_w * g2[None, :]) / 6.0).T).astype(BFD)
    fc1bc = np.ascontiguousarray(
        ((fc1_b + fc1_w @ b2) / 6.0 + 0.5).reshape(MCH, 128).T)
    diag6 = np.zeros((MCH, 9, 128, 128), np.float32)
    for m in range(MCH):
        for t_i, (di, dj) in enumerate(PE_TAPS):
            np.fill_diagonal(diag6[m, t_i],
                             conv_w[m * 128:(m + 1) * 128, 0, di + 1, dj + 1])
    diag6 = diag6.astype(BFD)
    convb6 = np.ascontiguousarray((conv_b / 6.0 + 0.5).reshape(MCH, 128).T)
    fc2T6 = np.ascontiguousarray((fc2_w * 6.0).T).astype(BFD)
    fc2b = fc2_b.reshape(1, C).astype(BFD)
    g1rep = np.broadcast_to(g1, (128, C)).copy()
    abrep = np.stack([np.broadcast_to(AREAS[s] * b1, (128, C))
                      for s in range(4)], axis=1).copy()
    agrep = np.broadcast_to(ag, (128, C)).astype(BFD).copy()
    btrep = np.broadcast_to(ab, (128, C)).astype(BFD).copy()
    poolw = np.zeros((128, CCH, 4, 9), np.float32)
    poolabd = np.zeros((128, CCH, 4), np.float32)
    for cc in range(CCH):
        for s in range(4):
            for tap in range(9):
                di, dj = TAPS[tap]
                poolw[:, cc, s, tap] = dconv_w[s, cc * 128:(cc + 1) * 128, 0,
                                               di + 1, dj + 1]
            poolabd[:, cc, s] = AREAS[s] * dconv_b[s, cc * 128:(cc + 1) * 128]
    onescolA = np.zeros((128, 64), np.float32)
    for h in range(NH):
        onescolA[:, h * 8 + h] = 1.0
    onescolB = np.zeros((128, 24), np.float32)
    for h in range(NH):
        t, hq = h // 3, h % 3
        onescolB[32 * hq:32 * hq + 13, 8 * t + h] = 1.0
    repmask2 = np.zeros((8, 512), np.float32)
    for p in range(4):
        repmask2[2 * p, p * 128:p * 128 + 64] = 1.0
        repmask2[2 * p + 1, p * 128 + 64:p * 128 + 128] = 1.0

    shared = dict(qgwT=qgwT, qb=qb, kwT=kwT, vwT=vwT, projTp=projTp,
                  projb=projb, fc1T6=fc1T6, fc1bc=fc1bc, diag6=diag6,
                  convb6=convb6, fc2T6=fc2T6, fc2b=fc2b,
                  g1rep=g1rep, abrep=abrep, agrep=agrep, btrep=btrep,
                  poolw=poolw, poolabd=poolabd,
                  onescolA=onescolA.astype(BFD), onescolB=onescolB.astype(BFD),
                  repmask2=repmask2,
                  identF=np.eye(128, dtype=np.float32),
                  identB=np.eye(128, dtype=np.float32).astype(BFD),
                  onesr=np.ones((1, 128), np.float32))

    perms = []
    for half in range(2):
        f0 = 0 if half == 0 else NTOK - TRNG
        perms.append(np.concatenate([np.arange(f0, f0 + TRNG),
                                     np.arange(0, f0),
                                     np.arange(f0 + TRNG, NTOK)]))
    masks_p = [np.ascontiguousarray(M[p]) for p in perms]

    in_maps = []
    for b in range(B):
        for half in range(2):
            m = dict(shared)
            m["x"] = np.ascontiguousarray(x[b][perms[half]])
            m["masks"] = masks_p[half]
            in_maps.append(m)
    return in_maps


def kernel(**inputs):
    if "nc" not in _cache:
        _cache["nc"] = build_program()
    nc = _cache["nc"]

    from concourse.bass_utils import run_bass_kernel_spmd

    in_maps = _host_prep(inputs)
    core_ids = list(range(8))
    res = run_bass_kernel_spmd(nc, in_maps, core_ids)

    x = np.asarray(inputs["x"], np.float32)
    out = np.empty_like(x)
    for b in range(B):
        o0 = res.results[2 * b]["out"]
        o1 = res.results[2 * b + 1]["out"]
        out[b, 0:HALF] = o0[0:HALF]
        out[b, HALF:] = o1[HALO:]
    return out
